# revision 4
# baseline (speedup 1.0000x reference)
"""
2D prefix-max kernel (bottom-pool then right-pool, i.e. cummax over H then W,
output doubled) for x[512, 256, 256] f32, sharded channel-wise over 8 cores.

out = 2 * cummax_w(cummax_h(x))

Per-core plan (64 channels):
  - Tiles of G=2 channels: load [128 (h-half), G*256 (c,w)] slabs.
  - W-cummax: tensor_tensor_scan along free dim, segmented per channel via a
    bias operand (-1e30 at each channel's w=0 column): state = max(bias +
    state, x).
  - PE transposes (128x128, fp32) into PSUM to put H in the free dim.
  - H-cummax: same segmented scan, reading PSUM directly.
  - PE transposes back to natural [h, (c,w)] layout in PSUM.
  - ScalarE activation copy with scale=2.0 PSUM->SBUF, DMA out.
"""

import numpy as np

from concourse import bacc, bass, mybir
from concourse.bass_utils import run_bass_kernel_spmd
from concourse.masks import make_identity
from concourse.tile import TileContext

C, H, W = 512, 256, 256
N_CORES = 8
C_PER = C // N_CORES  # 64 channels per core
G = 2  # channels per processing group (PSUM tile = [128, G*256] f32 = 1 bank)
NEG = -1e30

FP32 = mybir.dt.float32

_nc_cache = {}


def _build(c_per: int = C_PER) -> bass.Bass:
    nc = bacc.Bacc()
    x_d = nc.declare_dram_parameter("x", [c_per, H, W], FP32, isOutput=False)
    o_d = nc.declare_dram_parameter("out", [c_per, H, W], FP32, isOutput=True)

    add = mybir.AluOpType.add
    mx = mybir.AluOpType.max

    with TileContext(nc) as tc:
        with (
            tc.tile_pool(name="const", bufs=1) as cpool,
            tc.tile_pool(name="sb", bufs=4) as sb,
            tc.tile_pool(name="ps", bufs=2, space="PSUM") as ps,
        ):
            ident = cpool.tile([128, 128], FP32)
            make_identity(nc, ident[:])
            # Segment-reset bias: -1e30 at each channel's first scan element
            # (free index c*W), 0 elsewhere. Shared by the W-scan (free =
            # (c, w)) and the H-scan (free = (c, h)) since both segment at
            # multiples of 256.
            bias = cpool.tile([128, G * W], FP32)
            nc.vector.memset(bias[:], 0.0)
            for c in range(G):
                nc.vector.memset(bias[:, c * W : c * W + 1], NEG)

            for g in range(c_per // G):
                c0 = g * G
                # ---- load + W-scan (layout: p = h within half, free = (c, w))
                xs = []
                for hh in range(2):
                    X = sb.tile([128, G * W], FP32, tag=f"X{hh}")
                    nc.sync.dma_start(
                        out=X[:].rearrange("h (c w) -> h c w", c=G),
                        in_=x_d[c0 : c0 + G, hh * 128 : (hh + 1) * 128, :].rearrange(
                            "c h w -> h c w"
                        ),
                    )
                    nc.vector.tensor_tensor_scan(
                        out=X[:], data0=bias[:], data1=X[:], initial=NEG,
                        op0=add, op1=mx,
                    )
                    xs.append(X)

                # ---- transpose to (p = w within half, free = (c, h)), H-scan
                ss = []
                for wh in range(2):
                    T = ps.tile([128, G * W], FP32, tag=f"T{wh}")
                    for c in range(G):
                        for hh in range(2):
                            nc.tensor.transpose(
                                T[:, c * W + hh * 128 : c * W + hh * 128 + 128],
                                xs[hh][:, c * W + wh * 128 : c * W + wh * 128 + 128],
                                ident[:],
                            )
                    S = sb.tile([128, G * W], FP32, tag=f"S{wh}")
                    nc.vector.tensor_tensor_scan(
                        out=S[:], data0=bias[:], data1=T[:], initial=NEG,
                        op0=add, op1=mx,
                    )
                    ss.append(S)

                # ---- transpose back to natural layout, x2, store
                for hh in range(2):
                    Z = ps.tile([128, G * W], FP32, tag=f"Z{hh}")
                    for c in range(G):
                        for wh in range(2):
                            nc.tensor.transpose(
                                Z[:, c * W + wh * 128 : c * W + wh * 128 + 128],
                                ss[wh][:, c * W + hh * 128 : c * W + hh * 128 + 128],
                                ident[:],
                            )
                    O = sb.tile([128, G * W], FP32, tag=f"O{hh}")
                    nc.scalar.activation(
                        O[:], Z[:], mybir.ActivationFunctionType.Copy, scale=2.0
                    )
                    nc.sync.dma_start(
                        out=o_d[c0 : c0 + G, hh * 128 : (hh + 1) * 128, :].rearrange(
                            "c h w -> h c w"
                        ),
                        in_=O[:].rearrange("h (c w) -> h c w", c=G),
                    )
    nc.finalize()
    return nc


def run(x: np.ndarray, trace: bool = False):
    """Run on 8 cores; returns (full_output, BassKernelResults)."""
    key = ("full", trace)
    if "full" not in _nc_cache:
        _nc_cache["full"] = _build()
    nc = _nc_cache["full"]
    in_maps = [
        {"x": np.ascontiguousarray(x[i * C_PER : (i + 1) * C_PER])}
        for i in range(N_CORES)
    ]
    res = run_bass_kernel_spmd(nc, in_maps, list(range(N_CORES)), trace=trace)
    out = np.concatenate([res.results[i]["out"] for i in range(N_CORES)], axis=0)
    return out, res


def kernel(x: np.ndarray) -> np.ndarray:
    out, _ = run(np.asarray(x), trace=False)
    return out
